# revision 1
# baseline (speedup 1.0000x reference)
# Trainium2 Bass/Tile kernel for nn_Decoder (dense transformer decoder layer).
#
# Shapes (hardcoded per problem spec): B=4, T=S=D=1024, H=16 (hd=64).
# Sharding: 8 cores = (batch b = core//2) x (T-half = core%2). Each core
# computes out1[b, t_block, :] and wvn[b, t_block, :] for its 512 rows,
# recomputing the batch-level tensors it needs (full-T K/V for causal
# self-attention, encoder K/V, tv norms).
#
# SPMD trick: one program runs on all 8 cores. Per-core differences (which
# t-block, causal structure) are pushed into the DATA: decoder rows are
# permuted so each core's own 512 rows come first, and the causal mask is
# supplied as per-core mask tiles (attention sums are invariant to key order).
#
# Perf structure (vs the earlier baseline):
#   - softmax normalization is DEFERRED and batched: the per-head ones-column
#     denominator rows are collected into den[16, TB]; one [16,TB] reciprocal
#     replaces 16 slow single-partition reciprocals, and partition-broadcast
#     happens via tiny K=16 select-matrix matmuls whose PSUM output is read
#     directly by the normalizing vector multiplies.  Nothing on the PE queue
#     waits on a vector reciprocal any more.
#   - attention heads are software-pipelined: scores(h+1) is issued before
#     AV(h) so the PE never sits through exp(h).
#   - the probs-mean accumulation (wacc) runs as deferred chains split across
#     the vector AND gpsimd engines, overlapping O2/LN/MLP compute.  E2 tiles
#     live in an 8-slot ring.
#   - all HBM tensors are pre-tiled host-side into [128, ...] partition-major
#     contiguous layouts (large DMA descriptors), and weight DMAs are issued
#     one stage ahead.
#   - MLP2 accumulates in column quarters (2 PSUM banks each) so wvn
#     transposes can interleave.
import numpy as np
import ml_dtypes

import concourse.bass as bass
import concourse.tile as tile
from concourse import bacc
from concourse import mybir
from concourse.bass_utils import run_bass_kernel_spmd
from concourse.masks import make_identity

F32 = mybir.dt.float32
BF16 = mybir.dt.bfloat16
AF = mybir.ActivationFunctionType
ALU = mybir.AluOpType

B, T, S, D, H = 4, 1024, 1024, 1024, 16
HD = D // H          # 64
TB = T // 2          # 512 rows per core
P = 128
NT = TB // P         # 4 t-subtiles
ND = D // P          # 8 d-tiles
NS = S // P          # 8 s-tiles
F4 = 4 * D           # 4096
NF4 = F4 // P        # 32
EPS = 1e-6
BF = np.dtype(ml_dtypes.bfloat16)

# heads whose wacc chain runs on gpsimd (rest on vector)
GSET = frozenset((2, 4, 6, 9, 11, 13, 15))
E2_RING = 7
INVB_RING = 6

_CACHE = {}


def _build_program():
    nc = bacc.Bacc("TRN2", target_bir_lowering=False, debug=False)

    def din(name, shape, dt):
        return nc.dram_tensor(name, list(shape), dt, kind="ExternalInput").ap()

    t = {}
    t["dec"] = din("dec", (P, ND, D), F32)       # permuted: own block first
    t["decb"] = din("decb", (P, NT, D), F32)     # own block + bout1' (residual)
    t["enc"] = din("enc", (P, ND, D), F32)
    t["mask"] = din("mask", (P, NS, TB), BF16)   # causal mask, permuted s order
    for n in ["wq1", "wk1", "wv1", "wo1", "wq2", "wk2", "wv2", "wo2", "wtv"]:
        t[n] = din(n, (P, ND, D), BF16)
    t["w1"] = din("w1", (8, P, ND, 512), BF16)   # MLP1, 512-col chunks
    t["w2"] = din("w2", (4, P, NF4, 256), BF16)  # MLP2, 256-col quarters
    for n, shp in [("bq1", (P, ND)), ("bq2", (P, ND)), ("b1", (P, NF4)),
                   ("tvb", (P, ND))]:
        t[n] = din(n, shp, F32)
    t["bo2row"] = din("bo2row", (1, D), BF16)
    t["bm2row"] = din("bm2row", (1, D), BF16)
    t["sel1"] = din("sel1", (16, ND, P), BF16)
    t["selh"] = din("selh", (16, H, P), BF16)

    t["out1"] = nc.dram_tensor("out1", [TB, D], F32, kind="ExternalOutput").ap()
    t["wvn"] = nc.dram_tensor("wvn", [TB, S], F32, kind="ExternalOutput").ap()

    with tile.TileContext(nc) as tc:
        _body(tc, t)
    nc.compile()
    return nc


def _body(tc, t):
    nc = tc.nc

    open_cms = []

    def open_pool(name, bufs=1, space="SBUF"):
        cm = tc.tile_pool(name=name, bufs=bufs, space=space)
        pool = cm.__enter__()
        open_cms.append(cm)
        return cm, pool

    def close(cm):
        open_cms.remove(cm)
        cm.__exit__(None, None, None)

    try:
        _stages(tc, nc, t, open_pool, close)
    finally:
        for cm in reversed(open_cms):
            cm.__exit__(None, None, None)


def _stages(tc, nc, t, open_pool, close):
    ts = bass.ts

    _, consts = open_pool("consts", 1)
    _, stats = open_pool("stats", 4)
    _, p_x = open_pool("p_x", 1)
    _, invp = open_pool("invp", 1)

    cm_pmm, pmm = open_pool("pmm", 2, "PSUM")
    cm_ptp, ptp = open_pool("ptp", 1, "PSUM")
    cm_psc, psc = open_pool("psc", 3, "PSUM")
    cm_pav, pav = open_pool("pav", 2, "PSUM")

    # ---------------- constants ----------------
    ident_bf = consts.tile([P, P], BF16, tag="idbf")
    make_identity(nc, ident_bf)
    ident_f32 = consts.tile([P, P], F32, tag="idf32")
    make_identity(nc, ident_f32)
    ones_row = consts.tile([1, P], BF16, tag="ones_row")
    nc.vector.memset(ones_row, 1.0)
    ones_col = consts.tile([P, 1], BF16, tag="ones_col")
    nc.vector.memset(ones_col, 1.0)
    eps_sb = consts.tile([P, 1], F32, tag="eps")
    nc.vector.memset(eps_sb, EPS)
    # sel1[k, fo, p] = 1 iff k == 2*fo + (p >= 64): pair-broadcast selector
    sel1 = consts.tile([16, ND, P], BF16, tag="sel1")
    nc.sync.dma_start(sel1, t["sel1"])
    # selh[k, h, p] = 1 iff k == h: full-broadcast selector
    selh = consts.tile([16, H, P], BF16, tag="selh")
    nc.sync.dma_start(selh, t["selh"])
    bq1_sb = consts.tile([P, ND], F32, tag="bq1")
    nc.sync.dma_start(bq1_sb, t["bq1"])
    bq2_sb = consts.tile([P, ND], F32, tag="bq2")
    nc.sync.dma_start(bq2_sb, t["bq2"])
    b1_sb = consts.tile([P, NF4], F32, tag="b1")
    nc.sync.dma_start(b1_sb, t["b1"])
    tvb_sb = consts.tile([P, ND], F32, tag="tvb")
    nc.sync.dma_start(tvb_sb, t["tvb"])
    bo2_sb = consts.tile([1, D], BF16, tag="bo2")
    nc.sync.dma_start(bo2_sb, t["bo2row"])
    bm2_sb = consts.tile([1, D], BF16, tag="bm2")
    nc.sync.dma_start(bm2_sb, t["bm2row"])
    tvn_col = consts.tile([P, NS], F32, tag="tvncol")

    x_sb = p_x.tile([P, NT, D], F32, tag="x")

    den1 = invp.tile([16, TB], F32, tag="den1")
    inv1b = invp.tile([16, TB], BF16, tag="inv1b")
    inv2b = invp.tile([16, TB], BF16, tag="inv2b")
    # inv2b is consumed as a full [16, TB] matmul operand while still being
    # written group-by-group -- zero the not-yet-written rows.
    nc.vector.memset(inv2b, 0.0)

    def ln_apply(src2d, dst, a):
        """LN (no affine) of src2d ([128,1024] f32) -> dst[:, a, :] bf16."""
        st = stats.tile([P, 2, 6], F32, tag="ln_st")
        nc.vector.bn_stats(st[:, 0, :], src2d[:, 0:512])
        nc.vector.bn_stats(st[:, 1, :], src2d[:, 512:1024])
        mv = stats.tile([P, 2], F32, tag="ln_mv")
        nc.vector.bn_aggr(mv, st)
        sd = stats.tile([P, 1], F32, tag="ln_sd")
        nc.scalar.activation(sd, mv[:, 1:2], AF.Sqrt, bias=eps_sb)
        nc.vector.reciprocal(sd, sd)
        nc.vector.tensor_scalar(
            out=dst[:, a, :], in0=src2d, scalar1=mv[:, 0:1],
            scalar2=sd, op0=ALU.subtract, op1=ALU.mult)

    def transpose_to(dst, src, n_row_tiles, n_col_tiles, dt_):
        """src [128, n_row_tiles, >=n_col_tiles*128] -> dst [128, n_col_tiles,
        n_row_tiles*128] (matrix transpose)."""
        ident = ident_f32 if dt_ == F32 else ident_bf
        for c in range(n_col_tiles):
            for g0 in range(0, n_row_tiles, 4):
                gn = min(4, n_row_tiles - g0)
                ps = ptp.tile([P, 4 * P], dt_, tag="tp")
                for j in range(gn):
                    nc.tensor.transpose(ps[:, ts(j, P)],
                                        src[:, g0 + j, ts(c, P)], ident)
                nc.vector.tensor_copy(out=dst[:, c, g0 * P:(g0 + gn) * P],
                                      in_=ps[:, 0:gn * P])

    # ================= Phase A: dec load + LN + transpose =================
    cm_p1a, p1a = open_pool("p1a", 1)          # wo ring + decb   [A..end]
    cm_p1b, p1b = open_pool("p1b", 1)          # q1T,k1T,v1a      [A..D]
    q1T = p1b.tile([P, ND, TB], BF16, tag="q1T")
    k1T = p1b.tile([P, ND, S], BF16, tag="k1T")
    v1a = p1b.tile([P, NS, H * (HD + 1)], BF16, tag="v1a")
    v1a4 = v1a[:].rearrange("p a (h c) -> p a h c", c=HD + 1)
    nc.vector.memset(v1a4[:, :, :, HD:HD + 1], 1.0)

    cm_pa, p_pa = open_pool("p_pa", 1)         # xdt, xde         [A..B]
    xhat_deT = p_pa.tile([P, ND, T], BF16, tag="xdt")
    xhat_de = p_pa.tile([P, ND, D], BF16, tag="xde")
    cm_wqkv1, p_wqkv1 = open_pool("p_wqkv1", 1)
    cm_dec, dec_pool = open_pool("dec_pool", 1)

    dec_tiles = []
    for a in range(2):
        dt_ = dec_pool.tile([P, D], F32, tag=f"dec{a}", name=f"dec_{a}")
        nc.sync.dma_start(dt_, t["dec"][:, a, :])
        dec_tiles.append(dt_)
    wq1 = p_wqkv1.tile([P, ND, D], BF16, tag="wq")
    nc.sync.dma_start(wq1, t["wq1"])

    for a in range(ND):
        ln_apply(dec_tiles[a], xhat_de, a)
        if a + 2 < ND:
            dt_ = dec_pool.tile([P, D], F32, tag=f"dec{a % 2}",
                                name=f"dec_{a + 2}")
            nc.sync.dma_start(dt_, t["dec"][:, a + 2, :])
            dec_tiles.append(dt_)
    wk1 = p_wqkv1.tile([P, ND, D], BF16, tag="wk")
    nc.sync.dma_start(wk1, t["wk1"])
    wv1 = p_wqkv1.tile([P, ND, D], BF16, tag="wv")
    nc.sync.dma_start(wv1, t["wv1"])
    transpose_to(xhat_deT, xhat_de, ND, ND, BF16)
    close(cm_dec)

    # prefetch: O1 weights + residual base (needed phase D)
    wo1 = p1a.tile([P, ND, D], BF16, tag="wo", name="wo1")
    nc.sync.dma_start(wo1, t["wo1"])
    decb_sb = p1a.tile([P, NT, D], F32, tag="decb")
    nc.sync.dma_start(decb_sb, t["decb"])

    # ================= Phase B: QKV1 =================
    def qkv_block(wq, wk, wv, q_src, kv_src, qT, kT, va4, bq_tile):
        # Q^T [f, t] own rows only
        for ft in range(ND):
            ps = pmm.tile([P, TB], F32, tag="mm")
            for k in range(ND):
                nc.tensor.matmul(ps, wq[:, k, ts(ft, P)], q_src[:, k, 0:TB],
                                 start=k == 0, stop=k == ND - 1)
            nc.vector.tensor_scalar_add(qT[:, ft, :], ps, bq_tile[:, ft:ft + 1])
        # K^T [f, s] full S
        for ft in range(ND):
            for sc in range(S // 512):
                ps = pmm.tile([P, TB], F32, tag="mm")
                for k in range(ND):
                    nc.tensor.matmul(ps, wk[:, k, ts(ft, P)],
                                     kv_src[:, k, ts(sc, 512)],
                                     start=k == 0, stop=k == ND - 1)
                nc.scalar.activation(kT[:, ft, ts(sc, 512)], ps, AF.Copy)
        # V [s, dv] full S; lhsT = activation^T tiles (stationary), rhs = wv
        for st_ in range(NS):
            for dc in range(D // 512):
                ps = pmm.tile([P, TB], F32, tag="mm")
                for k in range(ND):
                    nc.tensor.matmul(ps, kv_src[:, k, ts(st_, P)],
                                     wv[:, k, ts(dc, 512)],
                                     start=k == 0, stop=k == ND - 1)
                nc.vector.tensor_copy(
                    out=va4[:, st_, dc * 8:(dc + 1) * 8, 0:HD],
                    in_=ps[:].rearrange("p (h c) -> p h c", c=HD))

    qkv_block(wq1, wk1, wv1, xhat_deT, xhat_deT, q1T, k1T, v1a4, bq1_sb)
    close(cm_wqkv1)
    close(cm_pa)

    # ================= Phase C: self-attention (deferred norm) ============
    cm_av1, p_av1 = open_pool("p_av1", 1)
    av_sb = p_av1.tile([P, ND, TB], BF16, tag="av")
    cm_mask, p_mask = open_pool("p_mask", 1)
    mask_sb = p_mask.tile([P, NS, TB], BF16, tag="mask")
    nc.sync.dma_start(mask_sb, t["mask"])
    cm_e1, e1_pool = open_pool("e1", 2)

    def scores_block(h, kT, qT, epool, etag, mask):
        fo, po = h // 2, (h % 2) * HD
        E = epool.tile([P, NS, TB], BF16, tag=etag, name=f"{etag}_{h}")
        for st_ in range(NS):
            ps = psc.tile([P, TB], F32, tag="sc")
            nc.tensor.matmul(ps, kT[po:po + HD, fo, ts(st_, P)],
                             qT[po:po + HD, fo, :], start=True, stop=True)
            nc.scalar.activation(E[:, st_, :], ps, AF.Exp)
            if mask is not None:
                nc.vector.tensor_mul(E[:, st_, :], E[:, st_, :],
                                     mask[:, st_, :])
        return E

    def av_block(h, E, va, av_out, dtmp, den_dst):
        fo, po = h // 2, (h % 2) * HD
        pa = pav.tile([HD + 1, TB], F32, tag="pav")
        for st_ in range(NS):
            nc.tensor.matmul(pa, va[:, st_, h * (HD + 1):(h + 1) * (HD + 1)],
                             E[:, st_, :], start=st_ == 0, stop=st_ == NS - 1)
        nc.vector.tensor_copy(av_out[po:po + HD, fo, :], pa[0:HD, :])
        # single-partition writes must start at an aligned partition: stage
        # the denominator row at partition 0, then DMA it into its slot.
        nc.vector.tensor_copy(dtmp, pa[HD:HD + 1, :])
        nc.sync.dma_start(den_dst, dtmp)

    def av1_block(h, E):
        dtmp = e1_pool.tile([1, TB], F32, tag="dt", name=f"dt1_{h}")
        av_block(h, E, v1a, av_sb, dtmp, den1[h:h + 1, :])

    prev = None
    for h in range(H):
        E = scores_block(h, k1T, q1T, e1_pool, "E1", mask_sb)
        if prev is not None:
            av1_block(prev[0], prev[1])
        prev = (h, E)
    av1_block(prev[0], prev[1])

    # epilogue: batched reciprocal + pair-broadcast + normalize
    nc.vector.reciprocal(den1, den1)
    nc.vector.tensor_copy(inv1b, den1)
    for fo in range(ND):
        ps = psc.tile([P, TB], F32, tag="sc")
        nc.tensor.matmul(ps, sel1[:, fo, :], inv1b, start=True, stop=True)
        nc.vector.tensor_mul(av_sb[:, fo, :], av_sb[:, fo, :], ps)
    close(cm_e1)
    close(cm_mask)

    # ================= Phase D: out-proj1 + residual -> x =================
    for tt in range(NT):
        for oc in range(D // 512):
            ps = pmm.tile([P, TB], F32, tag="mm")
            for ft in range(ND):
                nc.tensor.matmul(ps, av_sb[:, ft, ts(tt, P)],
                                 wo1[:, ft, ts(oc, 512)],
                                 start=ft == 0, stop=ft == ND - 1)
            nc.vector.tensor_add(x_sb[:, tt, ts(oc, 512)], ps,
                                 decb_sb[:, tt, ts(oc, 512)])
    close(cm_av1)
    close(cm_p1b)

    # O2 weights ride the wo ring slot: DMA starts once O1 reads are done
    wo2 = p1a.tile([P, ND, D], BF16, tag="wo", name="wo2")
    nc.sync.dma_start(wo2, t["wo2"])

    # long-lived accumulators for attn2 / MLP
    cm_acc, p_acc = open_pool("p_acc", 1)
    accV = p_acc.tile([P, NS, TB], F32, tag="accV")
    accG = p_acc.tile([P, NS, TB], BF16, tag="accG")
    lnxT = p_acc.tile([P, ND, TB], BF16, tag="lnxT")
    av2_sb = p_acc.tile([P, ND, TB], BF16, tag="av2")
    nc.vector.memset(accV, 0.0)
    nc.gpsimd.memset(accG, 0.0)

    cm_p2, p_p2 = open_pool("p_p2", 1)
    q2T = p_p2.tile([P, ND, TB], BF16, tag="q2T")
    k2T = p_p2.tile([P, ND, S], BF16, tag="k2T")
    v2a = p_p2.tile([P, NS, H * (HD + 1)], BF16, tag="v2a")
    v2a4 = v2a[:].rearrange("p a (h c) -> p a h c", c=HD + 1)
    nc.vector.memset(v2a4[:, :, :, HD:HD + 1], 1.0)

    # ================= Phase E: xT, enc LN, tv norms ======================
    cm_ent, p_ent = open_pool("p_ent", 1)
    xhat_enT = p_ent.tile([P, ND, S], BF16, tag="ent")
    xT = p_ent.tile([P, ND, TB], BF16, tag="xT")
    transpose_to(xT, x_sb, NT, ND, F32)

    cm_xen, p_xen = open_pool("p_xen", 1)
    xhat_en = p_xen.tile([P, ND, D], BF16, tag="xen")
    en_tiles = []
    for a in range(2):
        et = p_xen.tile([P, D], F32, tag=f"en{a}", name=f"en_{a}")
        nc.sync.dma_start(et, t["enc"][:, a, :])
        en_tiles.append(et)
    for a in range(ND):
        ln_apply(en_tiles[a], xhat_en, a)
        if a + 2 < ND:
            et = p_xen.tile([P, D], F32, tag=f"en{a % 2}", name=f"en_{a + 2}")
            nc.sync.dma_start(et, t["enc"][:, a + 2, :])
            en_tiles.append(et)
    transpose_to(xhat_enT, xhat_en, ND, ND, BF16)
    close(cm_xen)

    # tv norms: tv^T = wtv.T @ xhat_en^T ; tvn = sqrt(sum_f tv^2)/H
    cm_wtv, p_wtv = open_pool("p_wtv", 1)
    wtv = p_wtv.tile([P, ND, D], BF16, tag="wtv")
    nc.sync.dma_start(wtv, t["wtv"])
    tvn_row = p_wtv.tile([1, S], F32, tag="tvr")
    for sc in range(S // 512):
        pn = pav.tile([1, 512], F32, tag="pav")
        for ft in range(ND):
            ps = pmm.tile([P, TB], F32, tag="mm")
            for k in range(ND):
                nc.tensor.matmul(ps, wtv[:, k, ts(ft, P)],
                                 xhat_enT[:, k, ts(sc, 512)],
                                 start=k == 0, stop=k == ND - 1)
            tvq = p_wtv.tile([P, 512], BF16, tag=f"tvq{ft % 2}",
                             name=f"tvq_{sc}_{ft}")
            nc.scalar.activation(tvq, ps, AF.Square, bias=tvb_sb[:, ft:ft + 1])
            nc.tensor.matmul(pn, ones_col, tvq, start=ft == 0, stop=ft == ND - 1)
        nc.scalar.activation(tvn_row[:, ts(sc, 512)], pn, AF.Sqrt,
                             scale=1.0 / (H * H))
    pcol = ptp.tile([P, NS], F32, tag="tp")
    for so in range(NS):
        nc.tensor.transpose(pcol[:, so:so + 1], tvn_row[0:1, ts(so, P)],
                            ident_f32[0:1, 0:1])
    nc.vector.tensor_copy(out=tvn_col, in_=pcol)
    close(cm_wtv)

    # ================= Phase F: QKV2 =================
    cm_wqkv2, p_wqkv2 = open_pool("p_wqkv2", 1)
    wq2 = p_wqkv2.tile([P, ND, D], BF16, tag="wA", name="wq2")
    nc.sync.dma_start(wq2, t["wq2"])
    wk2 = p_wqkv2.tile([P, ND, D], BF16, tag="wB", name="wk2")
    nc.sync.dma_start(wk2, t["wk2"])
    wv2 = p_wqkv2.tile([P, ND, D], BF16, tag="wA", name="wv2")
    nc.sync.dma_start(wv2, t["wv2"])

    qkv_block(wq2, wk2, wv2, xT, xhat_enT, q2T, k2T, v2a4, bq2_sb)
    close(cm_wqkv2)
    close(cm_ent)

    # ================= Phase G: cross-attention + wacc ====================
    # per-2-head groups: batched reciprocal, PE broadcast, Pt = E*inv on
    # vector, head-sum adds mostly on gpsimd (accG) with a few on vector.
    cm_g2, p_g2 = open_pool("p_g2", 1)
    invb2 = p_g2.tile([P, 2, TB], BF16, tag="invb2")
    VADD = frozenset((5, 11))   # heads whose acc add runs on vector

    e2_tiles = {}
    dg_tiles = {}

    def emit_group2(g2):
        a = 2 * g2
        dg = dg_tiles[g2]
        nc.vector.reciprocal(dg, dg)
        ibt = p_g2.tile([2, TB], BF16, tag=f"ib{g2 % 2}", name=f"ib_{g2}")
        nc.vector.tensor_copy(ibt, dg)
        nc.sync.dma_start(inv2b[a:a + 2, :], ibt)
        ps = psc.tile([P, TB], F32, tag="sc")
        nc.tensor.matmul(ps, sel1[:, g2, :], inv2b, start=True, stop=True)
        nc.vector.tensor_mul(av2_sb[:, g2, :], av2_sb[:, g2, :], ps)
        for hh in (a, a + 1):
            ps2 = psc.tile([P, TB], F32, tag="sc")
            nc.tensor.matmul(ps2, selh[:, hh, :], inv2b, start=True, stop=True)
            nc.scalar.activation(invb2[:, hh % 2, :], ps2, AF.Copy)
            Pt = p_g2.tile([P, NS, TB], BF16, tag=f"pt{hh % 3}",
                           name=f"pt_{hh}")
            E = e2_tiles[hh]
            for st_ in range(NS):
                nc.vector.tensor_mul(Pt[:, st_, :], E[:, st_, :],
                                     invb2[:, hh % 2, :])
            if hh in VADD:
                nc.vector.tensor_add(accV[:, :, :], accV[:, :, :],
                                     Pt[:, :, :])
            else:
                nc.gpsimd.tensor_add(accG[:, :, :], accG[:, :, :],
                                     Pt[:, :, :])

    def av2_block(h):
        g2 = h // 2
        if h % 2 == 0:
            dg_tiles[g2] = p_g2.tile([2, TB], F32, tag=f"dg{g2 % 2}",
                                     name=f"dg_{g2}")
        dtmp = p_g2.tile([1, TB], F32, tag=f"dt{h % 2}", name=f"dt2_{h}")
        av_block(h, e2_tiles[h], v2a, av2_sb, dtmp,
                 dg_tiles[g2][h % 2:h % 2 + 1, :])

    prev = None
    for h in range(H):
        E = scores_block(h, k2T, q2T, p_g2, f"e2{h % 3}", None)
        e2_tiles[h] = E
        if prev is not None:
            av2_block(prev)
        prev = h
        if h >= 2 and h % 2 == 0:
            emit_group2(h // 2 - 1)
    av2_block(15)
    emit_group2(7)
    close(cm_g2)
    close(cm_p2)

    # ================= Phase H: out-proj2 + residual ======================
    for tt in range(NT):
        for oc in range(D // 512):
            ps = pmm.tile([P, TB], F32, tag="mm")
            for ft in range(ND):
                nc.tensor.matmul(ps, av2_sb[:, ft, ts(tt, P)],
                                 wo2[:, ft, ts(oc, 512)],
                                 start=ft == 0, stop=False)
            nc.tensor.matmul(ps, ones_row, bo2_sb[:, ts(oc, 512)],
                             start=False, stop=True)
            nc.vector.tensor_add(x_sb[:, tt, ts(oc, 512)], ps,
                                 x_sb[:, tt, ts(oc, 512)])

    # merge wacc halves and scale by tv norms
    nc.vector.tensor_add(accV[:, :, :], accV[:, :, :], accG[:, :, :])
    for so in range(NS):
        nc.vector.tensor_scalar_mul(accV[:, so, :], accV[:, so, :],
                                    tvn_col[:, so:so + 1])

    # ================= Phase I: LN(x2) -> lnxT; wvn out ===================
    cm_wvn, p_wvn = open_pool("p_wvn", 1)
    lnx = p_wvn.tile([P, NT, D], BF16, tag="lnx")
    for a in range(NT):
        ln_apply(x_sb[:, a, :], lnx, a)
    transpose_to(lnxT, lnx, NT, ND, BF16)

    for g in range(2):
        for tt in range(NT):
            ps = ptp.tile([P, 4 * P], F32, tag="tp")
            for j in range(4):
                nc.tensor.transpose(ps[:, ts(j, P)],
                                    accV[:, g * 4 + j, ts(tt, P)], ident_f32)
            ob = p_wvn.tile([P, 512], F32, tag=f"wst{tt % 2}",
                            name=f"wst_{g}_{tt}")
            nc.vector.tensor_copy(out=ob, in_=ps)
            nc.sync.dma_start(t["wvn"][ts(tt, P), g * 512:(g + 1) * 512], ob)
    close(cm_wvn)

    # ================= Phase J: MLP1 ======================================
    cm_hT, p_hT = open_pool("p_hT", 1)
    hT = p_hT.tile([P, NF4, TB], BF16, tag="hT")
    cm_w1, p_w1 = open_pool("p_w1", 1)

    w1_tiles = []
    for c in range(2):
        w1c = p_w1.tile([P, ND, 512], BF16, tag=f"w1{c % 2}", name=f"w1c_{c}")
        nc.sync.dma_start(w1c, t["w1"][c])
        w1_tiles.append(w1c)
    for c in range(8):
        w1c = w1_tiles[c]
        for ot in range(4):
            o = c * 4 + ot
            ps = pmm.tile([P, TB], F32, tag="mm")
            for k in range(ND):
                nc.tensor.matmul(ps, w1c[:, k, ts(ot, P)], lnxT[:, k, :],
                                 start=k == 0, stop=k == ND - 1)
            nc.scalar.activation(hT[:, o, :], ps, AF.Gelu,
                                 bias=b1_sb[:, o:o + 1])
        if c + 2 < 8:
            nx = p_w1.tile([P, ND, 512], BF16, tag=f"w1{c % 2}",
                           name=f"w1c_{c + 2}")
            nc.sync.dma_start(nx, t["w1"][c + 2])
            w1_tiles.append(nx)
    close(cm_w1)

    # ================= Phase K: MLP2 (column quarters) + out1 =============
    close(cm_pav)
    close(cm_psc)
    cm_pff, pff = open_pool("pff", 4, "PSUM")
    cm_w2, p_w2 = open_pool("p_w2", 1)

    w2_tiles = []
    for q in range(2):
        w2q = p_w2.tile([P, NF4, 256], BF16, tag=f"w2{q % 2}", name=f"w2q_{q}")
        nc.sync.dma_start(w2q, t["w2"][q])
        w2_tiles.append(w2q)
    for q in range(4):
        w2q = w2_tiles[q]
        ffs = [pff.tile([P, 512], F32, tag="ff", name=f"ff_{q}_{tt}")
               for tt in range(NT)]
        for k in range(NF4):
            for tt in range(NT):
                nc.tensor.matmul(ffs[tt][:, 0:256], hT[:, k, ts(tt, P)],
                                 w2q[:, k, :], start=k == 0, stop=False)
        for tt in range(NT):
            nc.tensor.matmul(ffs[tt][:, 0:256], ones_row,
                             bm2_sb[:, q * 256:(q + 1) * 256],
                             start=False, stop=True)
            ob = p_w2.tile([P, 256], F32, tag=f"st{tt % 2}",
                           name=f"st_{q}_{tt}")
            nc.vector.tensor_add(ob, ffs[tt][:, 0:256],
                                 x_sb[:, tt, q * 256:(q + 1) * 256])
            nc.sync.dma_start(t["out1"][ts(tt, P), q * 256:(q + 1) * 256], ob)
        if q + 2 < 4:
            nx = p_w2.tile([P, NF4, 256], BF16, tag=f"w2{q % 2}",
                           name=f"w2q_{q + 2}")
            nc.sync.dma_start(nx, t["w2"][q + 2])
            w2_tiles.append(nx)
    close(cm_w2)
    close(cm_pff)
    close(cm_hT)


def _mk_sel1():
    s = np.zeros((16, ND, P), np.float32)
    for fo in range(ND):
        for j in range(2):
            s[2 * fo + j, fo, j * HD:(j + 1) * HD] = 1.0
    return np.ascontiguousarray(s.astype(BF))


def _mk_selh():
    s = np.zeros((16, H, P), np.float32)
    for h in range(H):
        s[h, h, :] = 1.0
    return np.ascontiguousarray(s.astype(BF))


def _tile_pm(x, n_tiles):
    """[n_tiles*P, F] row-major -> [P, n_tiles, F] contiguous."""
    f = x.shape[1]
    return np.ascontiguousarray(x.reshape(n_tiles, P, f).transpose(1, 0, 2))


def _host_prep(inputs):
    """Fold LN affine + biases into weights; build per-core input maps."""
    f32 = np.float32
    g = np.asarray(inputs["ln_g"], f32)
    b = np.asarray(inputs["ln_b"], f32)
    w_in1 = np.asarray(inputs["w_in1"], f32)
    b_in1 = np.asarray(inputs["b_in1"], f32)
    w_out1 = np.asarray(inputs["w_out1"], f32)
    b_out1 = np.asarray(inputs["b_out1"], f32)
    w_in2 = np.asarray(inputs["w_in2"], f32)
    b_in2 = np.asarray(inputs["b_in2"], f32)
    w_out2 = np.asarray(inputs["w_out2"], f32)
    b_out2 = np.asarray(inputs["b_out2"], f32)
    mlp_w1 = np.asarray(inputs["mlp_w1"], f32)
    mlp_b1 = np.asarray(inputs["mlp_b1"], f32)
    mlp_w2 = np.asarray(inputs["mlp_w2"], f32)
    mlp_b2 = np.asarray(inputs["mlp_b2"], f32)
    dec = np.asarray(inputs["decoder_input"], f32)
    enc = np.asarray(inputs["encoder_output"], f32)

    wq1, wk1, wv1 = w_in1[:D], w_in1[D:2 * D], w_in1[2 * D:]
    wq2, wk2, wv2 = w_in2[:D], w_in2[D:2 * D], w_in2[2 * D:]
    sc = 1.0 / np.sqrt(HD)

    def bft(x):
        return _tile_pm(np.ascontiguousarray(x).astype(BF), ND)

    w1T = (mlp_w1 * g).T          # [D, F4]
    w2T = mlp_w2.T                # [F4, D]
    w1_chunks = np.stack([_tile_pm(w1T[:, c * 512:(c + 1) * 512].astype(BF), ND)
                          for c in range(8)])
    w2_quarts = np.stack(
        [np.ascontiguousarray(
            w2T[:, q * 256:(q + 1) * 256].astype(BF)
            .reshape(NF4, P, 256).transpose(1, 0, 2))
         for q in range(4)])

    shared = {
        "wq1": bft(((wq1 * g) * sc).T),
        "wk1": bft((wk1 * g).T),
        "wv1": bft((wv1 * g).T),
        "wo1": bft(w_out1.T),
        "wq2": bft((wq2 * sc).T),           # query = x (no LN)
        "wk2": bft((wk2 * g).T),
        "wv2": bft((wv2 * g).T),
        "wo2": bft(w_out2.T),
        "wtv": bft(w_out2 * g[:, None]),
        "w1": w1_chunks,
        "w2": w2_quarts,
        "bq1": np.ascontiguousarray(
            ((b_in1[:D] + wq1 @ b) * sc).reshape(ND, P).T.astype(f32)),
        "bq2": np.ascontiguousarray(
            ((b_in2[:D]) * sc).reshape(ND, P).T.astype(f32)),
        "b1": np.ascontiguousarray(
            (mlp_b1 + mlp_w1 @ b).reshape(NF4, P).T.astype(f32)),
        "tvb": np.ascontiguousarray(
            (b @ w_out2).reshape(ND, P).T.astype(f32)),
        "bo2row": np.ascontiguousarray(
            (b_out2 + w_out2 @ (b_in2[2 * D:] + wv2 @ b))[None, :].astype(BF)),
        "bm2row": np.ascontiguousarray(mlp_b2[None, :].astype(BF)),
        "sel1": _mk_sel1(),
        "selh": _mk_selh(),
    }
    bout1p = b_out1 + w_out1 @ (b_in1[2 * D:] + wv1 @ b)

    in_maps = []
    for c in range(8):
        bi, half = c // 2, c % 2
        t0 = half * TB
        perm = np.concatenate([np.arange(t0, t0 + TB),
                               np.arange(0, t0) if half else np.arange(TB, T)])
        m = perm[:, None] <= (t0 + np.arange(TB))[None, :]
        im = dict(shared)
        im["dec"] = _tile_pm(np.ascontiguousarray(dec[bi][perm]), ND)
        im["decb"] = _tile_pm(
            np.ascontiguousarray(dec[bi, t0:t0 + TB] + bout1p[None, :]), NT)
        im["enc"] = _tile_pm(np.ascontiguousarray(enc[bi]), ND)
        im["mask"] = _tile_pm(np.ascontiguousarray(m.astype(BF)), NS)
        in_maps.append(im)
    return in_maps


def run_sharded(inputs, trace=False, **kw):
    if "nc" not in _CACHE:
        _CACHE["nc"] = _build_program()
    nc = _CACHE["nc"]
    in_maps = _host_prep(inputs)
    res = run_bass_kernel_spmd(nc, in_maps, core_ids=list(range(8)),
                               trace=trace, **kw)
    out1 = np.zeros((B, T, D), np.float32)
    wvn = np.zeros((B, T, S), np.float32)
    for c in range(8):
        bi, half = c // 2, c % 2
        t0 = half * TB
        out1[bi, t0:t0 + TB] = res.results[c]["out1"]
        wvn[bi, t0:t0 + TB] = res.results[c]["wvn"]
    return (out1, wvn), res


def kernel(**inputs):
    outs, _ = run_sharded(inputs, trace=False)
    return outs



# revision 23
# speedup vs baseline: 1.1818x; 1.1818x over previous
# Trainium2 Bass/Tile kernel for nn_Decoder (dense transformer decoder layer).
#
# Shapes (hardcoded per problem spec): B=4, T=S=D=1024, H=16 (hd=64).
# Sharding: 8 cores = (batch b = core//2) x (T-half = core%2). Each core
# computes out1[b, t_block, :] and wvn[b, t_block, :] for its 512 rows,
# recomputing the batch-level tensors it needs (full-T K/V for causal
# self-attention, encoder K/V, tv norms).
#
# SPMD trick: one program runs on all 8 cores. Per-core differences (which
# t-block, causal structure) are pushed into the DATA: decoder rows are
# permuted so each core's own 512 rows come first, and the causal mask is
# supplied as per-core mask tiles (attention sums are invariant to key order).
#
# Perf structure (v2): the HAM clock gate halves the PE clock whenever the
# PE has no ~3.4us sustained-busy window, so every phase must keep the PE
# dense.  The two attention phases are packed with independent dense work:
#   - enc LN runs on vector during QKV1 (PE dense), so xhat_enT is ready
#     before self-attention starts.
#   - the tv-norm matmuls (wtv.T @ xhat_enT) run as filler chunks
#     interleaved into self-attention phase C.
#   - V2's second half (heads 8-15) runs as filler inside cross-attention
#     phase G; K2/V2(dc0)/Q2 form a dense block between C and G.
#   - the probs-mean accumulation is a pair tree: per head-pair Pt sums on
#     vector, two running chains on gpsimd, one merge + tvn scale at the
#     end (replaces the 14-deep serial gpsimd chain).
#   - softmax normalization stays deferred/batched via select-matrix
#     matmuls (PE broadcast) as in v1.
import numpy as np
import ml_dtypes

import concourse.bass as bass
import concourse.tile as tile
from concourse import bacc
from concourse import mybir
from concourse.bass_utils import run_bass_kernel_spmd
from concourse.masks import make_identity

F32 = mybir.dt.float32
BF16 = mybir.dt.bfloat16
AF = mybir.ActivationFunctionType
ALU = mybir.AluOpType

B, T, S, D, H = 4, 1024, 1024, 1024, 16
HD = D // H          # 64
TB = T // 2          # 512 rows per core
P = 128
NT = TB // P         # 4 t-subtiles
ND = D // P          # 8 d-tiles
NS = S // P          # 8 s-tiles
F4 = 4 * D           # 4096
NF4 = F4 // P        # 32
EPS = 1e-6
BF = np.dtype(ml_dtypes.bfloat16)

_CACHE = {}


def _build_program():
    nc = bacc.Bacc("TRN2", target_bir_lowering=False, debug=False)

    def din(name, shape, dt):
        return nc.dram_tensor(name, list(shape), dt, kind="ExternalInput").ap()

    t = {}
    t["dec"] = din("dec", (P, ND, D), F32)       # permuted: own block first
    t["decb"] = din("decb", (P, NT, D), F32)     # own block + bout1' (residual)
    t["enc"] = din("enc", (P, ND, D), F32)
    t["mask"] = din("mask", (P, NS, TB), BF16)   # causal mask, permuted s order
    for n in ["wq1", "wk1", "wv1", "wo1", "wq2", "wk2", "wv2", "wo2", "wtv"]:
        t[n] = din(n, (P, ND, D), BF16)
    t["w1"] = din("w1", (8, P, ND, 512), BF16)   # MLP1, 512-col chunks
    t["w2"] = din("w2", (4, P, NF4, 256), BF16)  # MLP2, 256-col quarters
    for n, shp in [("bq1", (P, ND)), ("bq2", (P, ND)), ("b1", (P, NF4)),
                   ("tvb", (P, ND))]:
        t[n] = din(n, shp, F32)
    t["bo2row"] = din("bo2row", (1, D), BF16)
    t["bm2row"] = din("bm2row", (1, D), BF16)
    t["sel1"] = din("sel1", (16, ND, P), BF16)
    t["selh"] = din("selh", (16, H, P), BF16)

    t["out1"] = nc.dram_tensor("out1", [TB, D], F32, kind="ExternalOutput").ap()
    t["wvn"] = nc.dram_tensor("wvn", [TB, S], F32, kind="ExternalOutput").ap()

    with tile.TileContext(nc) as tc:
        _body(tc, t)
    nc.compile()
    return nc


def _body(tc, t):
    nc = tc.nc

    open_cms = []

    def open_pool(name, bufs=1, space="SBUF", side=None):
        cm = tc.tile_pool(name=name, bufs=bufs, space=space, side=side)
        pool = cm.__enter__()
        open_cms.append(cm)
        return cm, pool

    def close(cm):
        open_cms.remove(cm)
        cm.__exit__(None, None, None)

    try:
        _stages(tc, nc, t, open_pool, close)
    finally:
        for cm in reversed(open_cms):
            cm.__exit__(None, None, None)


def _stages(tc, nc, t, open_pool, close):
    ts = bass.ts

    def open_pool_r(name, bufs=1):
        return open_pool(name, bufs, "SBUF", "right")

    _, consts = open_pool("consts", 1)
    _, stats = open_pool("stats", 4)
    _, p_x = open_pool("p_x", 1)
    _, invp = open_pool("invp", 1)

    cm_pmm, pmm = open_pool("pmm", 2, "PSUM")
    cm_ptp, ptp = open_pool("ptp", 1, "PSUM")
    cm_psc, psc = open_pool("psc", 3, "PSUM")
    cm_pav, pav = open_pool("pav", 2, "PSUM")

    # ---------------- constants ----------------
    ident_bf = consts.tile([P, P], BF16, tag="idbf")
    make_identity(nc, ident_bf)
    ident_f32 = consts.tile([P, P], F32, tag="idf32")
    make_identity(nc, ident_f32)
    ones_row = consts.tile([1, P], BF16, tag="ones_row")
    nc.vector.memset(ones_row, 1.0)
    ones_col = consts.tile([P, 1], BF16, tag="ones_col")
    nc.vector.memset(ones_col, 1.0)
    eps_sb = consts.tile([P, 1], F32, tag="eps")
    nc.vector.memset(eps_sb, EPS)
    # sel1[k, fo, p] = 1 iff k == 2*fo + (p >= 64): pair-broadcast selector
    sel1 = consts.tile([16, ND, P], BF16, tag="sel1")
    nc.sync.dma_start(sel1, t["sel1"])
    # selh[k, h, p] = 1 iff k == h: full-broadcast selector
    selh = consts.tile([16, H, P], BF16, tag="selh")
    nc.sync.dma_start(selh, t["selh"])
    bq1_sb = consts.tile([P, ND], F32, tag="bq1")
    nc.sync.dma_start(bq1_sb, t["bq1"])
    bq2_sb = consts.tile([P, ND], F32, tag="bq2")
    nc.sync.dma_start(bq2_sb, t["bq2"])
    b1_sb = consts.tile([P, NF4], F32, tag="b1")
    nc.sync.dma_start(b1_sb, t["b1"])
    tvb_sb = consts.tile([P, ND], F32, tag="tvb")
    nc.sync.dma_start(tvb_sb, t["tvb"])
    bo2_sb = consts.tile([1, D], BF16, tag="bo2")
    nc.sync.dma_start(bo2_sb, t["bo2row"])
    bm2_sb = consts.tile([1, D], BF16, tag="bm2")
    nc.sync.dma_start(bm2_sb, t["bm2row"])
    tvn_col = consts.tile([P, NS], F32, tag="tvncol")
    tvsq_col = consts.tile([P, NS], F32, tag="tvsq")

    x_sb = p_x.tile([P, NT, D], F32, tag="x")

    inv2b = invp.tile([16, TB], BF16, tag="inv2b")
    # inv2b is consumed as a full [16, TB] matmul operand while still being
    # written group-by-group -- zero the not-yet-written rows.
    nc.vector.memset(inv2b, 0.0)

    def ln_apply(src2d, dst, a):
        """LN (no affine) of src2d ([128,1024] f32) -> dst[:, a, :] bf16."""
        st = stats.tile([P, 2, 6], F32, tag="ln_st")
        nc.vector.bn_stats(st[:, 0, :], src2d[:, 0:512])
        nc.vector.bn_stats(st[:, 1, :], src2d[:, 512:1024])
        mv = stats.tile([P, 2], F32, tag="ln_mv")
        nc.vector.bn_aggr(mv, st)
        sd = stats.tile([P, 1], F32, tag="ln_sd")
        nc.scalar.activation(sd, mv[:, 1:2], AF.Sqrt, bias=eps_sb)
        nc.vector.reciprocal(sd, sd)
        nc.vector.tensor_scalar(
            out=dst[:, a, :], in0=src2d, scalar1=mv[:, 0:1],
            scalar2=sd, op0=ALU.subtract, op1=ALU.mult)

    def transpose_rows(dst, src, g0, gn, n_col_tiles, dt_):
        """transpose src row tiles [g0, g0+gn) into dst columns."""
        ident = ident_f32 if dt_ == F32 else ident_bf
        for c in range(n_col_tiles):
            ps = ptp.tile([P, 4 * P], dt_, tag="tp")
            for j in range(gn):
                nc.tensor.transpose(ps[:, ts(j, P)],
                                    src[:, g0 + j, ts(c, P)], ident)
            nc.vector.tensor_copy(out=dst[:, c, g0 * P:(g0 + gn) * P],
                                  in_=ps[:, 0:gn * P])

    def transpose_to(dst, src, n_row_tiles, n_col_tiles, dt_):
        for g0 in range(0, n_row_tiles, 4):
            gn = min(4, n_row_tiles - g0)
            transpose_rows(dst, src, g0, gn, n_col_tiles, dt_)

    # ================= Phase A: dec load + LN + transpose =================
    # right stack: pools that span awkward phase ranges
    cm_ent, p_ent = open_pool_r("p_ent")       # xhat_enT         [A..G]
    xhat_enT = p_ent.tile([P, ND, S], BF16, tag="ent")
    cm_wtv, p_wtv = open_pool_r("p_wtv")       # tv weights       [A..C]
    wtv = p_wtv.tile([P, ND, D], BF16, tag="wtv")
    nc.sync.dma_start(wtv, t["wtv"])
    tvn_row = p_wtv.tile([1, S], F32, tag="tvr")
    cm_xen, p_xen = open_pool_r("p_xen")       # xhat_en + enc    [A..B]
    xhat_en = p_xen.tile([P, ND, D], BF16, tag="xen")

    cm_p1b, p1b = open_pool("p1b", 1)          # q1T,k1T,v1a      [A..D]
    q1T = p1b.tile([P, ND, TB], BF16, tag="q1T")
    k1T = p1b.tile([P, ND, S], BF16, tag="k1T")
    v1a = p1b.tile([P, NS, H * (HD + 1)], BF16, tag="v1a")
    v1a4 = v1a[:].rearrange("p a (h c) -> p a h c", c=HD + 1)
    nc.vector.memset(v1a4[:, :, :, HD:HD + 1], 1.0)

    cm_pa, p_pa = open_pool("p_pa", 1)         # xhat_deT         [A..B]
    xhat_deT = p_pa.tile([P, ND, T], BF16, tag="xdt")
    cm_wqkv1, p_wqkv1 = open_pool("p_wqkv1", 1)   # 2-slot weight ring
    cm_de, p_de = open_pool("p_de", 1)         # xhat_de          [A only]
    xhat_de = p_de.tile([P, ND, D], BF16, tag="xde")
    cm_dec, dec_pool = open_pool("dec_pool", 1)

    dec_tiles = []
    for a in range(2):
        dt_ = dec_pool.tile([P, D], F32, tag=f"dec{a}", name=f"dec_{a}")
        nc.sync.dma_start(dt_, t["dec"][:, a, :])
        dec_tiles.append(dt_)
    wq1 = p_wqkv1.tile([P, ND, D], BF16, tag="wA", name="wq1")
    nc.sync.dma_start(wq1, t["wq1"])

    for a in range(ND):
        ln_apply(dec_tiles[a], xhat_de, a)
        if a + 2 < ND:
            dt_ = dec_pool.tile([P, D], F32, tag=f"dec{a % 2}",
                                name=f"dec_{a + 2}")
            nc.sync.dma_start(dt_, t["dec"][:, a + 2, :])
            dec_tiles.append(dt_)
        if a == 3:
            transpose_rows(xhat_deT, xhat_de, 0, 4, ND, BF16)
    wk1 = p_wqkv1.tile([P, ND, D], BF16, tag="wB", name="wk1")
    nc.sync.dma_start(wk1, t["wk1"])
    transpose_rows(xhat_deT, xhat_de, 4, 4, ND, BF16)
    close(cm_dec)
    close(cm_de)

    # enc tiles: LN on vector runs during QKV1 (PE dense)
    en_tiles = []
    for a in range(2):
        et = p_xen.tile([P, D], F32, tag=f"en{a}", name=f"en_{a}")
        nc.sync.dma_start(et, t["enc"][:, a, :])
        en_tiles.append(et)

    # ================= Phase B: QKV1 (+ enc LN on vector) =================
    en_state = {"a": 0}

    def enc_ln_hook(i):
        if i % 2 == 0 and en_state["a"] < ND:
            a = en_state["a"]
            ln_apply(en_tiles[a], xhat_en, a)
            if a + 2 < ND:
                et = p_xen.tile([P, D], F32, tag=f"en{a % 2}",
                                name=f"en_{a + 2}")
                nc.sync.dma_start(et, t["enc"][:, a + 2, :])
                en_tiles.append(et)
            en_state["a"] = a + 1

    # Q^T [f, t] own rows only
    for ft in range(ND):
        ps = pmm.tile([P, TB], F32, tag="mm")
        for k in range(ND):
            nc.tensor.matmul(ps, wq1[:, k, ts(ft, P)], xhat_deT[:, k, 0:TB],
                             start=k == 0, stop=k == ND - 1)
        nc.vector.tensor_scalar_add(q1T[:, ft, :], ps, bq1_sb[:, ft:ft + 1])
        enc_ln_hook(ft)
    # wv1 rides the wA slot once the Q matmuls are done
    wv1 = p_wqkv1.tile([P, ND, D], BF16, tag="wA", name="wv1")
    nc.sync.dma_start(wv1, t["wv1"])
    # K^T [f, s] full S
    for ft in range(ND):
        for sc in range(S // 512):
            ps = pmm.tile([P, TB], F32, tag="mm")
            for k in range(ND):
                nc.tensor.matmul(ps, wk1[:, k, ts(ft, P)],
                                 xhat_deT[:, k, ts(sc, 512)],
                                 start=k == 0, stop=k == ND - 1)
            nc.scalar.activation(k1T[:, ft, ts(sc, 512)], ps, AF.Copy)
        enc_ln_hook(ND + ft)
    # V [s, dv] full S; lhsT = activation^T tiles (stationary), rhs = wv
    for st_ in range(NS):
        for dc in range(D // 512):
            ps = pmm.tile([P, TB], F32, tag="mm")
            for k in range(ND):
                nc.tensor.matmul(ps, xhat_deT[:, k, ts(st_, P)],
                                 wv1[:, k, ts(dc, 512)],
                                 start=k == 0, stop=k == ND - 1)
            nc.vector.tensor_copy(
                out=v1a4[:, st_, dc * 8:(dc + 1) * 8, 0:HD],
                in_=ps[:].rearrange("p (h c) -> p h c", c=HD))
    close(cm_wqkv1)
    close(cm_pa)

    # xhat_enT transposes (PE) at the tail of phase B
    transpose_to(xhat_enT, xhat_en, ND, ND, BF16)
    close(cm_xen)

    # ================= Phase C: self-attention (deferred norm) ============
    # PE filler: tv-norm chunks (one (sc, ft) chunk each) interleaved
    # between attention heads.
    cm_p1a, p1a = open_pool("p1a", 1)          # wo1              [C..D]
    wo1 = p1a.tile([P, ND, D], BF16, tag="wo", name="wo1")
    nc.sync.dma_start(wo1, t["wo1"])
    cm_av1, p_av1 = open_pool("p_av1", 1)
    av_sb = p_av1.tile([P, ND, TB], BF16, tag="av")
    cm_den1, p_den1 = open_pool("p_den1", 1)   # den1/inv1b       [C only]
    den1 = p_den1.tile([16, TB], F32, tag="den1")
    inv1b = p_den1.tile([16, TB], BF16, tag="inv1b")
    cm_mask, p_mask = open_pool("p_mask", 1)
    mask_sb = p_mask.tile([P, NS, TB], BF16, tag="mask")
    nc.sync.dma_start(mask_sb, t["mask"])
    cm_e1, e1_pool = open_pool("e1", 2)

    # residual base (dec rows + folded bias) lands directly in x_sb
    nc.sync.dma_start(x_sb, t["decb"])

    tv_state = {"i": 0, "pn": None}

    def tv_chunk():
        i = tv_state["i"]
        if i >= 16:
            return
        tv_state["i"] = i + 1
        sc, ft = i // 8, i % 8
        if ft == 0:
            tv_state["pn"] = ptp.tile([1, 512], F32, tag="tp",
                                      name=f"tvpn_{sc}")
        pn = tv_state["pn"]
        ps = pmm.tile([P, TB], F32, tag="mm")
        for k in range(ND):
            nc.tensor.matmul(ps, wtv[:, k, ts(ft, P)],
                             xhat_enT[:, k, ts(sc, 512)],
                             start=k == 0, stop=k == ND - 1)
        tvq = p_wtv.tile([P, 512], BF16, tag=f"tvq{ft % 2}",
                         name=f"tvq_{sc}_{ft}")
        nc.scalar.activation(tvq, ps, AF.Square, bias=tvb_sb[:, ft:ft + 1])
        nc.tensor.matmul(pn, ones_col, tvq, start=ft == 0, stop=ft == ND - 1)
        if ft == ND - 1:
            nc.vector.tensor_copy(out=tvn_row[:, ts(sc, 512)], in_=pn)

    def scores_block(h, kT, qT, epool, etag, mask):
        fo, po = h // 2, (h % 2) * HD
        E = epool.tile([P, NS, TB], BF16, tag=etag, name=f"{etag}_{h}")
        for st_ in range(NS):
            ps = psc.tile([P, TB], F32, tag="sc")
            nc.tensor.matmul(ps, kT[po:po + HD, fo, ts(st_, P)],
                             qT[po:po + HD, fo, :], start=True, stop=True)
            nc.scalar.activation(E[:, st_, :], ps, AF.Exp)
            if mask is not None:
                nc.vector.tensor_mul(E[:, st_, :], E[:, st_, :],
                                     mask[:, st_, :])
        return E

    def av_block(h, E, va, av_out, dtmp, den_dst):
        fo, po = h // 2, (h % 2) * HD
        pa = pav.tile([HD + 1, TB], F32, tag="pav")
        for st_ in range(NS):
            nc.tensor.matmul(pa, va[:, st_, h * (HD + 1):(h + 1) * (HD + 1)],
                             E[:, st_, :], start=st_ == 0, stop=st_ == NS - 1)
        nc.vector.tensor_copy(av_out[po:po + HD, fo, :], pa[0:HD, :])
        # single-partition writes must start at an aligned partition: stage
        # the denominator row at partition 0, then DMA it into its slot.
        nc.vector.tensor_copy(dtmp, pa[HD:HD + 1, :])
        nc.sync.dma_start(den_dst, dtmp)

    def av1_block(h, E):
        dtmp = e1_pool.tile([1, TB], F32, tag="dt", name=f"dt1_{h}")
        av_block(h, E, v1a, av_sb, dtmp, den1[h:h + 1, :])

    prev = None
    for h in range(H):
        tv_chunk()
        E = scores_block(h, k1T, q1T, e1_pool, "E1", mask_sb)
        if prev is not None:
            av1_block(prev[0], prev[1])
        prev = (h, E)
    av1_block(prev[0], prev[1])

    # epilogue: batched reciprocal + pair-broadcast + normalize
    nc.vector.reciprocal(den1, den1)
    nc.vector.tensor_copy(inv1b, den1)
    for fo in range(ND):
        tv_chunk()
        ps = psc.tile([P, TB], F32, tag="sc")
        nc.tensor.matmul(ps, sel1[:, fo, :], inv1b, start=True, stop=True)
        nc.vector.tensor_mul(av_sb[:, fo, :], av_sb[:, fo, :], ps)
    while tv_state["i"] < 16:
        tv_chunk()
    close(cm_e1)
    close(cm_mask)
    close(cm_den1)

    # tv sum-of-squares -> column layout (sqrt deferred to phase I)
    pcol = ptp.tile([P, NS], F32, tag="tp")
    for so in range(NS):
        nc.tensor.transpose(pcol[:, so:so + 1], tvn_row[0:1, ts(so, P)],
                            ident_f32[0:1, 0:1])
    nc.vector.tensor_copy(out=tvsq_col, in_=pcol)
    close(cm_wtv)

    # ================= Phase D: out-proj1 + residual -> x =================
    for tt in range(NT):
        for oc in range(D // 512):
            ps = pmm.tile([P, TB], F32, tag="mm")
            for ft in range(ND):
                nc.tensor.matmul(ps, av_sb[:, ft, ts(tt, P)],
                                 wo1[:, ft, ts(oc, 512)],
                                 start=ft == 0, stop=ft == ND - 1)
            nc.vector.tensor_add(x_sb[:, tt, ts(oc, 512)], ps,
                                 x_sb[:, tt, ts(oc, 512)])
    close(cm_av1)
    close(cm_p1a)
    close(cm_p1b)

    # O2 weights (own pool; footprint starts after wo1 frees)
    cm_wo2, p_wo2 = open_pool("p_wo2", 1)
    wo2 = p_wo2.tile([P, ND, D], BF16, tag="wo2")
    nc.sync.dma_start(wo2, t["wo2"])

    # long-lived tiles for attn2 / wvn / MLP
    cm_acc, p_acc = open_pool("p_acc", 1)
    av2_sb = p_acc.tile([P, ND, TB], BF16, tag="av2")
    runA = p_acc.tile([P, NS, TB], BF16, tag="runA")   # wvn chain A / merged
    runB = p_acc.tile([P, NS, TB], BF16, tag="runB")   # wvn chain B

    cm_p2, p_p2 = open_pool("p_p2", 1)
    q2T = p_p2.tile([P, ND, TB], BF16, tag="q2T")
    k2T = p_p2.tile([P, ND, S], BF16, tag="k2T")
    v2a = p_p2.tile([P, NS, H * (HD + 1)], BF16, tag="v2a")
    v2a4 = v2a[:].rearrange("p a (h c) -> p a h c", c=HD + 1)
    nc.vector.memset(v2a4[:, :, :, HD:HD + 1], 1.0)

    # ================= Phase E': K2 + V2(dc0) + xT + Q2 ===================
    cm_wv2, p_wv2 = open_pool("p_wv2", 1)      # wv2 lives into G (dc1 filler)
    wv2 = p_wv2.tile([P, ND, D], BF16, tag="wv2")
    nc.sync.dma_start(wv2, t["wv2"])
    cm_wqkv2, p_wqkv2 = open_pool("p_wqkv2", 1)
    wk2 = p_wqkv2.tile([P, ND, D], BF16, tag="wB", name="wk2")
    nc.sync.dma_start(wk2, t["wk2"])

    # K2 full S
    for ft in range(ND):
        for sc in range(S // 512):
            ps = pmm.tile([P, TB], F32, tag="mm")
            for k in range(ND):
                nc.tensor.matmul(ps, wk2[:, k, ts(ft, P)],
                                 xhat_enT[:, k, ts(sc, 512)],
                                 start=k == 0, stop=k == ND - 1)
            nc.scalar.activation(k2T[:, ft, ts(sc, 512)], ps, AF.Copy)

    # xT transposes (PE) -- x is ready from phase D
    cm_xt, p_xt = open_pool("p_xt", 1)
    xT = p_xt.tile([P, ND, TB], BF16, tag="xT")
    transpose_to(xT, x_sb, NT, ND, F32)

    # V2 (dc0: heads 0..7)
    def v2_chunk(st_, dc):
        ps = pmm.tile([P, TB], F32, tag="mm")
        for k in range(ND):
            nc.tensor.matmul(ps, xhat_enT[:, k, ts(st_, P)],
                             wv2[:, k, ts(dc, 512)],
                             start=k == 0, stop=k == ND - 1)
        nc.vector.tensor_copy(
            out=v2a4[:, st_, dc * 8:(dc + 1) * 8, 0:HD],
            in_=ps[:].rearrange("p (h c) -> p h c", c=HD))

    for st_ in range(NS):
        v2_chunk(st_, 0)

    # wq2 rides the wB slot once the K2 matmuls are done
    wq2 = p_wqkv2.tile([P, ND, D], BF16, tag="wB", name="wq2")
    nc.sync.dma_start(wq2, t["wq2"])

    # Q2 (needs xT)
    for ft in range(ND):
        ps = pmm.tile([P, TB], F32, tag="mm")
        for k in range(ND):
            nc.tensor.matmul(ps, wq2[:, k, ts(ft, P)], xT[:, k, 0:TB],
                             start=k == 0, stop=k == ND - 1)
        nc.vector.tensor_scalar_add(q2T[:, ft, :], ps, bq2_sb[:, ft:ft + 1])
    close(cm_xt)
    close(cm_wqkv2)

    # ================= Phase G: cross-attention + wvn tree ================
    # per-2-head groups: batched reciprocal, PE broadcast, Pt = E*inv and
    # pair sums on vector; two running chains on gpsimd; merge + tvn scale
    # in phase I.  V2's dc1 half runs as PE filler during early heads.
    cm_g2, p_g2 = open_pool("p_g2", 1)
    invb2 = p_g2.tile([P, 2, TB], BF16, tag="invb2")

    e2_tiles = {}
    dg_tiles = {}
    pr_tiles = {}
    v2_state = {"i": 0}

    def v2_filler():
        i = v2_state["i"]
        if i < NS:
            v2_state["i"] = i + 1
            v2_chunk(i, 1)

    def emit_group2(g2):
        a = 2 * g2
        dg = dg_tiles.pop(g2)
        nc.vector.reciprocal(dg, dg)
        ibt = p_g2.tile([2, TB], BF16, tag="ib", name=f"ib_{g2}")
        nc.vector.tensor_copy(ibt, dg)
        nc.sync.dma_start(inv2b[a:a + 2, :], ibt)
        ps = psc.tile([P, TB], F32, tag="sc")
        nc.tensor.matmul(ps, sel1[:, g2, :], inv2b, start=True, stop=True)
        nc.vector.tensor_mul(av2_sb[:, g2, :], av2_sb[:, g2, :], ps)
        # Pt for both heads of the pair, then the pair sum (vector)
        pts = []
        for hh in (a, a + 1):
            ps2 = psc.tile([P, TB], F32, tag="sc")
            nc.tensor.matmul(ps2, selh[:, hh, :], inv2b, start=True, stop=True)
            nc.scalar.activation(invb2[:, hh % 2, :], ps2, AF.Copy)
            Pt = p_g2.tile([P, NS, TB], BF16, tag=f"pt{hh % 2}",
                           name=f"pt_{hh}")
            E = e2_tiles.pop(hh)
            for st_ in range(NS):
                nc.vector.tensor_mul(Pt[:, st_, :], E[:, st_, :],
                                     invb2[:, hh % 2, :])
            pts.append(Pt)
        pr = p_g2.tile([P, NS, TB], BF16, tag=f"pr{g2 % 2}", name=f"pr_{g2}")
        nc.vector.tensor_add(pr[:, :, :], pts[0][:, :, :], pts[1][:, :, :])
        pr_tiles[g2] = pr
        # running chains on gpsimd: runA accumulates pairs 0-3, runB 4-7
        run = runA if g2 < 4 else runB
        if g2 % 4 == 1:
            nc.gpsimd.tensor_add(run[:, :, :],
                                 pr_tiles.pop(g2 - 1)[:, :, :],
                                 pr_tiles.pop(g2)[:, :, :])
        elif g2 % 4 != 0:
            nc.gpsimd.tensor_add(run[:, :, :], run[:, :, :],
                                 pr_tiles.pop(g2)[:, :, :])

    def av2_block(h):
        g2 = h // 2
        if h % 2 == 0:
            dg_tiles[g2] = p_g2.tile([2, TB], F32, tag=f"dg{g2 % 2}",
                                     name=f"dg_{g2}")
        dtmp = p_g2.tile([1, TB], F32, tag="dt", name=f"dt2_{h}")
        av_block(h, e2_tiles[h], v2a, av2_sb, dtmp,
                 dg_tiles[g2][h % 2:h % 2 + 1, :])

    prev = None
    for h in range(H):
        v2_filler()
        E = scores_block(h, k2T, q2T, p_g2, f"e2{h % 3}", None)
        e2_tiles[h] = E
        if prev is not None:
            av2_block(prev)
            if prev % 2 == 1:
                emit_group2(prev // 2)
        prev = h
    av2_block(15)
    emit_group2(7)

    # ================= Phase H: out-proj2 + residual ======================
    for tt in range(NT):
        for oc in range(D // 512):
            ps = pmm.tile([P, TB], F32, tag="mm")
            for ft in range(ND):
                nc.tensor.matmul(ps, av2_sb[:, ft, ts(tt, P)],
                                 wo2[:, ft, ts(oc, 512)],
                                 start=ft == 0, stop=False)
            nc.tensor.matmul(ps, ones_row, bo2_sb[:, ts(oc, 512)],
                             start=False, stop=True)
            nc.vector.tensor_add(x_sb[:, tt, ts(oc, 512)], ps,
                                 x_sb[:, tt, ts(oc, 512)])

    # merge the two running chains (in place into runA)
    nc.vector.tensor_add(runA[:, :, :], runA[:, :, :], runB[:, :, :])
    close(cm_g2)
    close(cm_wv2)
    close(cm_p2)
    close(cm_ent)

    # ================= Phase I: LN(x2) -> lnxT; wvn out ===================
    cm_lnxT, p_lnxT = open_pool("p_lnxT", 1)   # lnxT             [I..K]
    lnxT = p_lnxT.tile([P, ND, TB], BF16, tag="lnxT")
    cm_wvn, p_wvn = open_pool("p_wvn", 1)
    lnx = p_wvn.tile([P, NT, D], BF16, tag="lnx")
    for a in range(NT):
        ln_apply(x_sb[:, a, :], lnx, a)
    transpose_to(lnxT, lnx, NT, ND, BF16)

    # tvn = sqrt(sum tv^2)/H (sqrt set is already loaded for ln_apply);
    # scale the merged probs-mean by tvn per s-tile.
    nc.scalar.activation(tvn_col, tvsq_col, AF.Sqrt, scale=1.0 / (H * H))
    for so in range(NS):
        nc.vector.tensor_scalar_mul(runA[:, so, :], runA[:, so, :],
                                    tvn_col[:, so:so + 1])

    for g in range(2):
        for tt in range(NT):
            ps = ptp.tile([P, 4 * P], BF16, tag="tp")
            for j in range(4):
                nc.tensor.transpose(ps[:, ts(j, P)],
                                    runA[:, g * 4 + j, ts(tt, P)], ident_bf)
            ob = p_wvn.tile([P, 512], F32, tag=f"wst{tt % 2}",
                            name=f"wst_{g}_{tt}")
            nc.vector.tensor_copy(out=ob, in_=ps)
            nc.sync.dma_start(t["wvn"][ts(tt, P), g * 512:(g + 1) * 512], ob)
    close(cm_wvn)

    # ================= Phase J: MLP1 ======================================
    cm_hT, p_hT = open_pool("p_hT", 1)
    hT = p_hT.tile([P, NF4, TB], BF16, tag="hT")
    cm_w1, p_w1 = open_pool("p_w1", 1)

    w1_tiles = []
    for c in range(2):
        w1c = p_w1.tile([P, ND, 512], BF16, tag=f"w1{c % 2}", name=f"w1c_{c}")
        nc.sync.dma_start(w1c, t["w1"][c])
        w1_tiles.append(w1c)
    for c in range(8):
        w1c = w1_tiles[c]
        for ot in range(4):
            o = c * 4 + ot
            ps = pmm.tile([P, TB], F32, tag="mm")
            for k in range(ND):
                nc.tensor.matmul(ps, w1c[:, k, ts(ot, P)], lnxT[:, k, :],
                                 start=k == 0, stop=k == ND - 1)
            nc.scalar.activation(hT[:, o, :], ps, AF.Gelu,
                                 bias=b1_sb[:, o:o + 1])
        if c + 2 < 8:
            nx = p_w1.tile([P, ND, 512], BF16, tag=f"w1{c % 2}",
                           name=f"w1c_{c + 2}")
            nc.sync.dma_start(nx, t["w1"][c + 2])
            w1_tiles.append(nx)
    close(cm_w1)

    # ================= Phase K: MLP2 (column quarters) + out1 =============
    close(cm_pav)
    close(cm_psc)
    cm_pff, pff = open_pool("pff", 4, "PSUM")
    cm_w2, p_w2 = open_pool("p_w2", 1)

    w2_tiles = []
    for q in range(2):
        w2q = p_w2.tile([P, NF4, 256], BF16, tag=f"w2{q % 2}", name=f"w2q_{q}")
        nc.sync.dma_start(w2q, t["w2"][q])
        w2_tiles.append(w2q)
    for q in range(4):
        w2q = w2_tiles[q]
        ffs = [pff.tile([P, 512], F32, tag="ff", name=f"ff_{q}_{tt}")
               for tt in range(NT)]
        for k in range(NF4):
            for tt in range(NT):
                nc.tensor.matmul(ffs[tt][:, 0:256], hT[:, k, ts(tt, P)],
                                 w2q[:, k, :], start=k == 0, stop=False)
        for tt in range(NT):
            nc.tensor.matmul(ffs[tt][:, 0:256], ones_row,
                             bm2_sb[:, q * 256:(q + 1) * 256],
                             start=False, stop=True)
            ob = p_w2.tile([P, 256], F32, tag=f"st{tt % 2}",
                           name=f"st_{q}_{tt}")
            nc.vector.tensor_add(ob, ffs[tt][:, 0:256],
                                 x_sb[:, tt, q * 256:(q + 1) * 256])
            nc.sync.dma_start(t["out1"][ts(tt, P), q * 256:(q + 1) * 256], ob)
        if q + 2 < 4:
            nx = p_w2.tile([P, NF4, 256], BF16, tag=f"w2{q % 2}",
                           name=f"w2q_{q + 2}")
            nc.sync.dma_start(nx, t["w2"][q + 2])
            w2_tiles.append(nx)
    close(cm_w2)
    close(cm_pff)
    close(cm_hT)


def _mk_sel1():
    s = np.zeros((16, ND, P), np.float32)
    for fo in range(ND):
        for j in range(2):
            s[2 * fo + j, fo, j * HD:(j + 1) * HD] = 1.0
    return np.ascontiguousarray(s.astype(BF))


def _mk_selh():
    s = np.zeros((16, H, P), np.float32)
    for h in range(H):
        s[h, h, :] = 1.0
    return np.ascontiguousarray(s.astype(BF))


def _tile_pm(x, n_tiles):
    """[n_tiles*P, F] row-major -> [P, n_tiles, F] contiguous."""
    f = x.shape[1]
    return np.ascontiguousarray(x.reshape(n_tiles, P, f).transpose(1, 0, 2))


def _host_prep(inputs):
    """Fold LN affine + biases into weights; build per-core input maps."""
    f32 = np.float32
    g = np.asarray(inputs["ln_g"], f32)
    b = np.asarray(inputs["ln_b"], f32)
    w_in1 = np.asarray(inputs["w_in1"], f32)
    b_in1 = np.asarray(inputs["b_in1"], f32)
    w_out1 = np.asarray(inputs["w_out1"], f32)
    b_out1 = np.asarray(inputs["b_out1"], f32)
    w_in2 = np.asarray(inputs["w_in2"], f32)
    b_in2 = np.asarray(inputs["b_in2"], f32)
    w_out2 = np.asarray(inputs["w_out2"], f32)
    b_out2 = np.asarray(inputs["b_out2"], f32)
    mlp_w1 = np.asarray(inputs["mlp_w1"], f32)
    mlp_b1 = np.asarray(inputs["mlp_b1"], f32)
    mlp_w2 = np.asarray(inputs["mlp_w2"], f32)
    mlp_b2 = np.asarray(inputs["mlp_b2"], f32)
    dec = np.asarray(inputs["decoder_input"], f32)
    enc = np.asarray(inputs["encoder_output"], f32)

    wq1, wk1, wv1 = w_in1[:D], w_in1[D:2 * D], w_in1[2 * D:]
    wq2, wk2, wv2 = w_in2[:D], w_in2[D:2 * D], w_in2[2 * D:]
    sc = 1.0 / np.sqrt(HD)

    def bft(x):
        return _tile_pm(np.ascontiguousarray(x).astype(BF), ND)

    w1T = (mlp_w1 * g).T          # [D, F4]
    w2T = mlp_w2.T                # [F4, D]
    w1_chunks = np.stack([_tile_pm(w1T[:, c * 512:(c + 1) * 512].astype(BF), ND)
                          for c in range(8)])
    w2_quarts = np.stack(
        [np.ascontiguousarray(
            w2T[:, q * 256:(q + 1) * 256].astype(BF)
            .reshape(NF4, P, 256).transpose(1, 0, 2))
         for q in range(4)])

    shared = {
        "wq1": bft(((wq1 * g) * sc).T),
        "wk1": bft((wk1 * g).T),
        "wv1": bft((wv1 * g).T),
        "wo1": bft(w_out1.T),
        "wq2": bft((wq2 * sc).T),           # query = x (no LN)
        "wk2": bft((wk2 * g).T),
        "wv2": bft((wv2 * g).T),
        "wo2": bft(w_out2.T),
        "wtv": bft(w_out2 * g[:, None]),
        "w1": w1_chunks,
        "w2": w2_quarts,
        "bq1": np.ascontiguousarray(
            ((b_in1[:D] + wq1 @ b) * sc).reshape(ND, P).T.astype(f32)),
        "bq2": np.ascontiguousarray(
            ((b_in2[:D]) * sc).reshape(ND, P).T.astype(f32)),
        "b1": np.ascontiguousarray(
            (mlp_b1 + mlp_w1 @ b).reshape(NF4, P).T.astype(f32)),
        "tvb": np.ascontiguousarray(
            (b @ w_out2).reshape(ND, P).T.astype(f32)),
        "bo2row": np.ascontiguousarray(
            (b_out2 + w_out2 @ (b_in2[2 * D:] + wv2 @ b))[None, :].astype(BF)),
        "bm2row": np.ascontiguousarray(mlp_b2[None, :].astype(BF)),
        "sel1": _mk_sel1(),
        "selh": _mk_selh(),
    }
    bout1p = b_out1 + w_out1 @ (b_in1[2 * D:] + wv1 @ b)

    in_maps = []
    for c in range(8):
        bi, half = c // 2, c % 2
        t0 = half * TB
        perm = np.concatenate([np.arange(t0, t0 + TB),
                               np.arange(0, t0) if half else np.arange(TB, T)])
        m = perm[:, None] <= (t0 + np.arange(TB))[None, :]
        im = dict(shared)
        im["dec"] = _tile_pm(np.ascontiguousarray(dec[bi][perm]), ND)
        im["decb"] = _tile_pm(
            np.ascontiguousarray(dec[bi, t0:t0 + TB] + bout1p[None, :]), NT)
        im["enc"] = _tile_pm(np.ascontiguousarray(enc[bi]), ND)
        im["mask"] = _tile_pm(np.ascontiguousarray(m.astype(BF)), NS)
        in_maps.append(im)
    return in_maps


def run_sharded(inputs, trace=False, **kw):
    if "nc" not in _CACHE:
        _CACHE["nc"] = _build_program()
    nc = _CACHE["nc"]
    in_maps = _host_prep(inputs)
    res = run_bass_kernel_spmd(nc, in_maps, core_ids=list(range(8)),
                               trace=trace, **kw)
    out1 = np.zeros((B, T, D), np.float32)
    wvn = np.zeros((B, T, S), np.float32)
    for c in range(8):
        bi, half = c // 2, c % 2
        t0 = half * TB
        out1[bi, t0:t0 + TB] = res.results[c]["out1"]
        wvn[bi, t0:t0 + TB] = res.results[c]["wvn"]
    return (out1, wvn), res


def kernel(**inputs):
    outs, _ = run_sharded(inputs, trace=False)
    return outs
